# revision 18
# baseline (speedup 1.0000x reference)
"""Causal GQA attention (B=2, L=2048, D=2048, H=16, KV=4, K=128) on 8 trn2 cores.

Sharding: core = b*4 + g  (b: batch 0..1, g: GQA group 0..3).
Each core computes, for its batch b and its 4 Q heads / 1 KV head:
    q/k/v projections -> rope -> causal attention -> partial out-projection
and writes yT_partial = (partial y).T to DRAM (bf16). Host sums the 4 group
partials per batch and transposes back.

v2 vs baseline:
 - all PE operands bf16 (halves DMA + SBUF traffic; LDW data)
 - x fully resident in SBUF; projections reordered (oc-pairs outer, d, l
   inner) so each weight-chunk stationary load serves 4 matmuls
 - wo resident (one DMA) instead of 256 per-jq wos DMAs
 - softmax row sums: DVE accumulates exp chunks, one ones-matmul per
   (head, lq-tile) instead of one per chunk (saves ~144 PE matmuls)
 - x DMA triggers on the (otherwise idle) gpsimd queue
"""

import sys

if "/opt/trn_rl_repo" not in sys.path:
    sys.path.insert(0, "/opt/trn_rl_repo")

import numpy as np

B, L, D, H, KV = 2, 2048, 2048, 16, 4
K = D // H          # 128 head dim
G = H // KV         # 4 q heads per kv head
NH = G              # q heads per core
LT = 512            # seq tile (moving operand width)
NLT = L // LT       # 4
ND = D // 128       # 16 contraction chunks
NJ = D // 128       # 16 output-column chunks
ROPE_BASE = 10000.0
MASK_VAL = -30000.0

_NC_CACHE = {}


def _build_nc():
    import concourse.bacc as bacc
    import concourse.mybir as mybir
    from concourse.tile import TileContext

    f32 = mybir.dt.float32
    f32r = mybir.dt.float32r
    bf16 = mybir.dt.bfloat16
    EXP = mybir.ActivationFunctionType.Exp
    nc = bacc.Bacc("TRN2", target_bir_lowering=False, debug=False, num_devices=8)

    # ---- DRAM parameters (host-pre-tiled layouts, bf16) ----
    xT = nc.dram_tensor("xT", [ND, 128, L], bf16, kind="ExternalInput")
    wqT = nc.dram_tensor("wqT", [ND, 128, 512], bf16, kind="ExternalInput")
    wkT = nc.dram_tensor("wkT", [ND, 128, 128], bf16, kind="ExternalInput")
    wvT = nc.dram_tensor("wvT", [ND, 128, 128], bf16, kind="ExternalInput")
    woT = nc.dram_tensor("woT", [128, NH * NJ * 128], bf16, kind="ExternalInput")
    cosT = nc.dram_tensor("cosT", [128, L], f32, kind="ExternalInput")
    sinT = nc.dram_tensor("sinT", [128, L], f32, kind="ExternalInput")
    masks = nc.dram_tensor("masks", [4, 128, LT], f32, kind="ExternalInput")
    pswap = nc.dram_tensor("pswap", [128, 128], bf16, kind="ExternalInput")
    onesc = nc.dram_tensor("onesc", [128, 8], bf16, kind="ExternalInput")
    ident = nc.dram_tensor("ident", [128, 128], bf16, kind="ExternalInput")
    yT = nc.dram_tensor("yT", [NJ, NLT, 128, LT], bf16, kind="ExternalOutput")

    with TileContext(nc) as tc:
        p_const = tc.alloc_tile_pool(name="const", bufs=1)
        p_wkv = tc.alloc_tile_pool(name="wkv", bufs=1)
        p_x = tc.alloc_tile_pool(name="xres", bufs=1)
        p_vraw = tc.alloc_tile_pool(name="vraw", bufs=1)
        p_rope = tc.alloc_tile_pool(name="ropeout", bufs=1)
        p_qs = tc.alloc_tile_pool(name="qs", bufs=4)
        p_tmp = tc.alloc_tile_pool(name="tmp", bufs=4)
        p_pt = tc.alloc_tile_pool(name="pt", bufs=3)
        p_acc = tc.alloc_tile_pool(name="acc", bufs=2)
        p_on = tc.alloc_tile_pool(name="on", bufs=2)
        p_rc = tc.alloc_tile_pool(name="rc", bufs=2)
        p_bc = tc.alloc_tile_pool(name="bc", bufs=2)
        p_ysb = tc.alloc_tile_pool(name="ysb", bufs=3)

        # ---- constants / weights (prefetch on sync queue) ----
        cos_sb = p_const.tile([128, L], f32, tag="cos", name="cos")
        nc.sync.dma_start(out=cos_sb[:], in_=cosT.ap())
        sin_sb = p_const.tile([128, L], f32, tag="sin", name="sin")
        nc.sync.dma_start(out=sin_sb[:], in_=sinT.ap())
        pswap_sb = p_const.tile([128, 128], bf16, tag="pswap", name="pswap")
        nc.sync.dma_start(out=pswap_sb[:], in_=pswap.ap())
        ones_sb = p_const.tile([128, 8], bf16, tag="ones", name="ones")
        nc.sync.dma_start(out=ones_sb[:], in_=onesc.ap())
        ident_sb = p_const.tile([128, 128], bf16, tag="ident", name="ident")
        nc.sync.dma_start(out=ident_sb[:], in_=ident.ap())
        mask_sb = p_const.tile([128, 1280], f32, tag="mask", name="mask")
        moff = [0, 128, 384, 768]
        for r in range(4):
            w = 128 * (r + 1)
            nc.sync.dma_start(out=mask_sb[:, moff[r]:moff[r] + w],
                              in_=masks.ap()[r][:, 0:w])

        wk_sb = p_wkv.tile([128, ND * 128], bf16, tag="wk", name="wk")
        wv_sb = p_wkv.tile([128, ND * 128], bf16, tag="wv", name="wv")
        wq_sb = p_wkv.tile([128, ND * 512], bf16, tag="wq", name="wq")
        wo_sb = p_wkv.tile([128, NH * NJ * 128], bf16, tag="wo", name="wo")
        for d in range(ND):
            nc.scalar.dma_start(out=wk_sb[:, d * 128:(d + 1) * 128],
                                in_=wkT.ap()[d])
            nc.scalar.dma_start(out=wq_sb[:, d * 512:(d + 1) * 512],
                                in_=wqT.ap()[d])
        for d in range(ND):
            nc.scalar.dma_start(out=wv_sb[:, d * 128:(d + 1) * 128],
                                in_=wvT.ap()[d])
        nc.sync.dma_start(out=wo_sb[:], in_=woT.ap())

        # ---- resident x: [128, d*2048 + l*512 + i], DMA'd d-major on gpsimd ----
        x_sb = p_x.tile([128, ND * L], bf16, tag="x", name="x")
        for d in range(ND):
            nc.gpsimd.dma_start(
                out=x_sb[:, d * L: (d + 1) * L], in_=xT.ap()[d])

        # rope outputs + v
        vrawT = p_vraw.tile([128, L], bf16, tag="vrawT", name="vrawT")
        qrope = [p_rope.tile([128, L], bf16, tag=f"qrope{h}", name=f"qrope{h}")
                 for h in range(NH)]
        krope = p_rope.tile([128, L], bf16, tag="krope", name="krope")
        v_sb = p_rope.tile([128, L], bf16, tag="v", name="v")

        # PSUM: psA holds 8 banks (2 oc-groups of 4 l-tiles)
        psA = tc.alloc_tile_pool(name="psA", bufs=1, space="PSUM")

        # ---- phase A: projections, oc-pairs outer / d / l inner ----
        # oc: 0=k, 1..4=q heads 0..3, 5=v
        rope_dst = [krope] + qrope
        for ocp in range(3):
            oc_pair = (2 * ocp, 2 * ocp + 1)
            ps_tiles = {}
            for oi, oc in enumerate(oc_pair):
                for l in range(NLT):
                    slot = oi * NLT + l
                    ps_tiles[(oc, l)] = psA.tile(
                        [128, LT], f32, tag=f"pj{slot}", name=f"pj{slot}")
            for d in range(ND):
                for oc in oc_pair:
                    if oc == 0:
                        w_ap = wk_sb[:, d * 128:(d + 1) * 128]
                    elif oc < 5:
                        qh = oc - 1
                        w_ap = wq_sb[:, d * 512 + qh * 128:d * 512 + (qh + 1) * 128]
                    else:
                        w_ap = wv_sb[:, d * 128:(d + 1) * 128]
                    for l in range(NLT):
                        nc.tensor.matmul(
                            ps_tiles[(oc, l)][:], w_ap,
                            x_sb[:, d * L + l * LT: d * L + (l + 1) * LT],
                            start=(d == 0), stop=(d == ND - 1))
            # rope / copy-out for this pair
            for oi, oc in enumerate(oc_pair):
                for l in range(NLT):
                    lsl = slice(l * LT, (l + 1) * LT)
                    slot = oi * NLT + l
                    if oc == 5:
                        nc.scalar.copy(vrawT[:, lsl], ps_tiles[(oc, l)][:])
                        continue
                    qs = p_qs.tile([128, LT], bf16, tag="qs", name="qs")
                    nc.scalar.copy(qs[:], ps_tiles[(oc, l)][:])
                    # swap matmul reuses the same PSUM slot (after qs copy)
                    psw = psA.tile([128, LT], f32, tag=f"pj{slot}",
                                   name=f"psw{slot}")
                    nc.tensor.matmul(psw[:], pswap_sb[:], qs[:],
                                     start=True, stop=True)
                    t1 = p_tmp.tile([128, LT], f32, tag="t1", name="t1")
                    nc.vector.tensor_mul(t1[:], qs[:], cos_sb[:, lsl])
                    t2 = p_tmp.tile([128, LT], f32, tag="t2", name="t2")
                    nc.vector.tensor_mul(t2[:], psw[:], sin_sb[:, lsl])
                    nc.vector.tensor_add(rope_dst[oc][:, lsl], t1[:], t2[:])
        psA.release()

        # ---- v transpose: v_sb[lk, kdim] in 16 column chunks ----
        psVT = tc.alloc_tile_pool(name="psVT", bufs=2, space="PSUM")
        for c in range(L // 128):
            pvt = psVT.tile([128, 128], bf16, tag="vt", name="vt")
            nc.tensor.transpose(pvt[:], vrawT[:, c * 128:(c + 1) * 128], ident_sb[:])
            nc.scalar.copy(v_sb[:, c * 128:(c + 1) * 128], pvt[:])
        psVT.release()

        # ---- phases C+D fused per lq-tile ----
        psS = tc.alloc_tile_pool(name="psS", bufs=2, space="PSUM")
        psO = tc.alloc_tile_pool(name="psO", bufs=2, space="PSUM")
        psSUM = tc.alloc_tile_pool(name="psSUM", bufs=2, space="PSUM")
        psY = tc.alloc_tile_pool(name="psY", bufs=2, space="PSUM")
        for jq in range(NLT):
            qsl = slice(jq * LT, (jq + 1) * LT)
            onorm = []
            for h in range(NH):
                nch = 4 * (jq + 1)
                po = psO.tile([128, LT], f32, tag="po", name="po")
                psm = psSUM.tile([1, LT], f32, tag="pm", name="pm")
                for c in range(nch):
                    ps = psS.tile([128, LT], f32, tag="ps", name="ps")
                    nc.tensor.matmul(ps[:], krope[:, c * 128:(c + 1) * 128],
                                     qrope[h][:, qsl], start=True, stop=True)
                    r = c - 4 * jq
                    if r >= 0:
                        w = 128 * (r + 1)
                        nc.vector.tensor_add(ps[:, 0:w], ps[:, 0:w],
                                             mask_sb[:, moff[r]:moff[r] + w])
                    pt = p_pt.tile([128, LT], bf16, tag="pt", name="pt")
                    nc.scalar.activation(pt[:], ps[:], EXP)
                    nc.tensor.matmul(po[:], v_sb[:, c * 128:(c + 1) * 128],
                                     pt[:], start=(c == 0),
                                     stop=(c == nch - 1), skip_group_check=True)
                    nc.tensor.matmul(psm[:], ones_sb[:, 0:1], pt[:],
                                     start=(c == 0), stop=(c == nch - 1),
                                     skip_group_check=True)
                sm = p_rc.tile([1, LT], f32, tag="rc", name="rc")
                nc.vector.tensor_copy(sm[:], psm[:])
                bs = p_bc.tile([128, LT], f32, tag="bs", name="bs")
                nc.gpsimd.partition_broadcast(bs[:], sm[:])
                bc = p_bc.tile([128, LT], f32, tag="bc", name="bc")
                nc.vector.reciprocal_approx_fast(bc[:], bs[:])
                on = p_on.tile([128, LT], bf16, tag=f"on{h}", name=f"on{h}")
                nc.vector.tensor_mul(on[:], po[:], bc[:])
                onorm.append(on)
            # output projection for this lq-tile (resident wo)
            for j in range(NJ):
                py = psY.tile([128, LT], f32, tag="py", name="py")
                for h in range(NH):
                    nc.tensor.matmul(
                        py[:], wo_sb[:, (h * NJ + j) * 128:(h * NJ + j + 1) * 128],
                        onorm[h][:], start=(h == 0), stop=(h == NH - 1))
                yt = p_ysb.tile([128, LT], bf16, tag="yt", name="yt")
                if j % 2 == 0:
                    nc.vector.tensor_copy(yt[:], py[:])
                else:
                    nc.scalar.copy(yt[:], py[:])
                nc.sync.dma_start(out=yT.ap()[j, jq], in_=yt[:])
        psY.release()
        psSUM.release()
        psO.release()
        psS.release()
        for pool in (p_ysb, p_bc, p_rc, p_on, p_acc, p_pt, p_tmp, p_qs,
                     p_rope, p_vraw, p_x, p_wkv, p_const):
            pool.release()

    nc.compile()
    return nc


def _get_nc():
    if "nc" not in _NC_CACHE:
        import concourse.mybir as mybir  # noqa: F401
        _NC_CACHE["nc"] = _build_nc()
    return _NC_CACHE["nc"]


def _host_tables():
    import ml_dtypes
    bf = ml_dtypes.bfloat16
    iv = (1.0 / (ROPE_BASE ** (np.arange(0, K, 2, dtype=np.float32) / np.float32(K)))).astype(np.float32)
    t = np.arange(L, dtype=np.float32)
    freqs = np.outer(t, iv).astype(np.float32)          # [L, 64]
    cos = np.cos(freqs).astype(np.float32)
    sin = np.sin(freqs).astype(np.float32)
    cosT = np.empty((128, L), np.float32)
    sinT = np.empty((128, L), np.float32)
    cosT[0::2] = cos.T
    cosT[1::2] = cos.T
    sinT[0::2] = -sin.T
    sinT[1::2] = sin.T

    p = np.arange(128)[:, None]
    f = np.arange(LT)[None, :]
    masks = np.zeros((4, 128, LT), np.float32)
    for r in range(4):
        masks[r] = np.where(f < 128 * r + p, np.float32(MASK_VAL), np.float32(0.0))

    pswap = np.zeros((128, 128), np.float32)
    idx = np.arange(128)
    pswap[idx ^ 1, idx] = 1.0
    onesc = np.ones((128, 8), np.float32)
    ident = np.eye(128, dtype=np.float32)
    return (cosT, sinT, masks, pswap.astype(bf), onesc.astype(bf),
            ident.astype(bf))


def _tile_xT(xb, bf):
    # x[b] [L, D] -> xT tiles [ND, 128, L]: xT[d, l] = x[l, d]
    xT = xb.T.astype(bf)  # [D, L]
    return np.ascontiguousarray(xT.reshape(ND, 128, L))


def _prep_inputs(x, wq, wk, wv, wo):
    import ml_dtypes
    bf = ml_dtypes.bfloat16
    cosT, sinT, masks, pswap, onesc, ident = _host_tables()
    scale = np.float32(K) ** np.float32(-0.5)
    in_maps = []
    xts = [_tile_xT(np.ascontiguousarray(x[b]), bf) for b in range(B)]
    for b in range(B):
        for g in range(KV):
            wq_g = (wq[g * 512:(g + 1) * 512, :] * scale).astype(bf)
            wqT_t = np.ascontiguousarray(
                wq_g.T.reshape(ND, 128, 512))                      # [d, 128, 512]
            wk_g = wk[g * 128:(g + 1) * 128, :].astype(bf)
            wkT_t = np.ascontiguousarray(wk_g.T.reshape(ND, 128, 128))
            wv_g = wv[g * 128:(g + 1) * 128, :].astype(bf)
            wvT_t = np.ascontiguousarray(wv_g.T.reshape(ND, 128, 128))
            wo_g = wo[:, g * 512:(g + 1) * 512]                    # [D, 512]
            # woT flat [128, (h*NJ+j)*128 + c] = wo[128j+c, 512g+128h+p]
            woT_t = np.ascontiguousarray(
                wo_g.T.reshape(NH, 128, NJ, 128).transpose(1, 0, 2, 3)
                .reshape(128, NH * NJ * 128)).astype(bf)
            in_maps.append({
                "xT": xts[b], "wqT": wqT_t, "wkT": wkT_t, "wvT": wvT_t,
                "woT": woT_t, "cosT": cosT, "sinT": sinT, "masks": masks,
                "pswap": pswap, "onesc": onesc, "ident": ident,
            })
    return in_maps


def _gather(results):
    out = np.empty((B, L, D), np.float32)
    for b in range(B):
        acc = None
        for g in range(KV):
            yt = results[b * KV + g]["yT"].astype(np.float32)  # [NJ, NLT, 128, LT]
            full = yt.transpose(0, 2, 1, 3).reshape(D, L)      # [j, l]
            acc = full if acc is None else acc + full
        out[b] = acc.T
    return out


def run(inputs, trace=False, trace_kwargs=None):
    from concourse.bass_utils import run_bass_kernel_spmd
    nc = _get_nc()
    in_maps = _prep_inputs(**inputs)
    res = run_bass_kernel_spmd(nc, in_maps, list(range(8)), trace=trace,
                               **(trace_kwargs or {}))
    return _gather(res.results), res


def kernel(x, wq, wk, wv, wo):
    out, _ = run({"x": x, "wq": wq, "wk": wk, "wv": wv, "wo": wo})
    return out


# revision 20
# speedup vs baseline: 1.1375x; 1.1375x over previous
"""Causal GQA attention (B=2, L=2048, D=2048, H=16, KV=4, K=128) on 8 trn2 cores.

Sharding: core = b*4 + g  (b: batch 0..1, g: GQA group 0..3).
Each core computes, for its batch b and its 4 Q heads / 1 KV head:
    q/k/v projections -> rope -> causal attention -> partial out-projection
and writes yT_partial = (partial y).T to DRAM (bf16). Host sums the 4 group
partials per batch and transposes back.

v2 vs baseline:
 - all PE operands bf16 (halves DMA + SBUF traffic; LDW data)
 - x fully resident in SBUF; projections reordered (oc-pairs outer, d, l
   inner) so each weight-chunk stationary load serves 4 matmuls
 - wo resident (one DMA) instead of 256 per-jq wos DMAs
 - softmax row sums: DVE accumulates exp chunks, one ones-matmul per
   (head, lq-tile) instead of one per chunk (saves ~144 PE matmuls)
 - x DMA triggers on the (otherwise idle) gpsimd queue
"""

import sys

if "/opt/trn_rl_repo" not in sys.path:
    sys.path.insert(0, "/opt/trn_rl_repo")

import numpy as np

B, L, D, H, KV = 2, 2048, 2048, 16, 4
K = D // H          # 128 head dim
G = H // KV         # 4 q heads per kv head
NH = G              # q heads per core
LT = 512            # seq tile (moving operand width)
NLT = L // LT       # 4
ND = D // 128       # 16 contraction chunks
NJ = D // 128       # 16 output-column chunks
ROPE_BASE = 10000.0
MASK_VAL = -30000.0

_NC_CACHE = {}


def _build_nc():
    import concourse.bacc as bacc
    import concourse.mybir as mybir
    from concourse.tile import TileContext

    f32 = mybir.dt.float32
    f32r = mybir.dt.float32r
    bf16 = mybir.dt.bfloat16
    EXP = mybir.ActivationFunctionType.Exp
    nc = bacc.Bacc("TRN2", target_bir_lowering=False, debug=False, num_devices=8)

    # ---- DRAM parameters (host-pre-tiled layouts, bf16) ----
    xT = nc.dram_tensor("xT", [ND, 128, L], bf16, kind="ExternalInput")
    wqT = nc.dram_tensor("wqT", [ND, 128, 512], bf16, kind="ExternalInput")
    wkT = nc.dram_tensor("wkT", [ND, 128, 128], bf16, kind="ExternalInput")
    wvT = nc.dram_tensor("wvT", [ND, 128, 128], bf16, kind="ExternalInput")
    woT = nc.dram_tensor("woT", [128, NH * NJ * 128], bf16, kind="ExternalInput")
    cosT = nc.dram_tensor("cosT", [128, L], f32, kind="ExternalInput")
    sinT = nc.dram_tensor("sinT", [128, L], f32, kind="ExternalInput")
    masks = nc.dram_tensor("masks", [4, 128, LT], f32, kind="ExternalInput")
    pswap = nc.dram_tensor("pswap", [128, 128], bf16, kind="ExternalInput")
    onesc = nc.dram_tensor("onesc", [128, 8], bf16, kind="ExternalInput")
    ident = nc.dram_tensor("ident", [128, 128], bf16, kind="ExternalInput")
    yT = nc.dram_tensor("yT", [NJ, NLT, 128, LT], bf16, kind="ExternalOutput")

    with TileContext(nc) as tc:
        p_const = tc.alloc_tile_pool(name="const", bufs=1)
        p_wkv = tc.alloc_tile_pool(name="wkv", bufs=1)
        p_x = tc.alloc_tile_pool(name="xres", bufs=1)
        p_vraw = tc.alloc_tile_pool(name="vraw", bufs=1)
        p_rope = tc.alloc_tile_pool(name="ropeout", bufs=1)
        p_qs = tc.alloc_tile_pool(name="qs", bufs=4)
        p_tmp = tc.alloc_tile_pool(name="tmp", bufs=4)
        p_pt = tc.alloc_tile_pool(name="pt", bufs=3)
        p_acc = tc.alloc_tile_pool(name="acc", bufs=2)
        p_on = tc.alloc_tile_pool(name="on", bufs=2)
        p_rc = tc.alloc_tile_pool(name="rc", bufs=2)
        p_bc = tc.alloc_tile_pool(name="bc", bufs=2)
        p_ysb = tc.alloc_tile_pool(name="ysb", bufs=3)

        # ---- constants / weights (prefetch on sync queue) ----
        cos_sb = p_const.tile([128, L], f32, tag="cos", name="cos")
        nc.sync.dma_start(out=cos_sb[:], in_=cosT.ap())
        sin_sb = p_const.tile([128, L], f32, tag="sin", name="sin")
        nc.sync.dma_start(out=sin_sb[:], in_=sinT.ap())
        pswap_sb = p_const.tile([128, 128], bf16, tag="pswap", name="pswap")
        nc.sync.dma_start(out=pswap_sb[:], in_=pswap.ap())
        ones_sb = p_const.tile([128, 8], bf16, tag="ones", name="ones")
        nc.sync.dma_start(out=ones_sb[:], in_=onesc.ap())
        ident_sb = p_const.tile([128, 128], bf16, tag="ident", name="ident")
        nc.sync.dma_start(out=ident_sb[:], in_=ident.ap())
        mask_sb = p_const.tile([128, 1280], f32, tag="mask", name="mask")
        moff = [0, 128, 384, 768]
        for r in range(4):
            w = 128 * (r + 1)
            nc.sync.dma_start(out=mask_sb[:, moff[r]:moff[r] + w],
                              in_=masks.ap()[r][:, 0:w])

        wk_sb = p_wkv.tile([128, ND * 128], bf16, tag="wk", name="wk")
        wv_sb = p_wkv.tile([128, ND * 128], bf16, tag="wv", name="wv")
        wq_sb = p_wkv.tile([128, ND * 512], bf16, tag="wq", name="wq")
        wo_sb = p_wkv.tile([128, NH * NJ * 128], bf16, tag="wo", name="wo")
        for d in range(ND):
            nc.scalar.dma_start(out=wk_sb[:, d * 128:(d + 1) * 128],
                                in_=wkT.ap()[d])
            nc.scalar.dma_start(out=wq_sb[:, d * 512:(d + 1) * 512],
                                in_=wqT.ap()[d])
        for d in range(ND):
            nc.scalar.dma_start(out=wv_sb[:, d * 128:(d + 1) * 128],
                                in_=wvT.ap()[d])
        nc.sync.dma_start(out=wo_sb[:], in_=woT.ap())

        # ---- resident x: [128, d*2048 + l*512 + i], DMA'd d-major on gpsimd ----
        x_sb = p_x.tile([128, ND * L], bf16, tag="x", name="x")
        for d in range(ND):
            nc.gpsimd.dma_start(
                out=x_sb[:, d * L: (d + 1) * L], in_=xT.ap()[d])

        # rope outputs + v
        vrawT = p_vraw.tile([128, L], bf16, tag="vrawT", name="vrawT")
        qrope = [p_rope.tile([128, L], bf16, tag=f"qrope{h}", name=f"qrope{h}")
                 for h in range(NH)]
        krope = p_rope.tile([128, L], bf16, tag="krope", name="krope")
        v_sb = p_rope.tile([128, L], bf16, tag="v", name="v")

        # PSUM: psA holds 8 banks (2 oc-groups of 4 l-tiles)
        psA = tc.alloc_tile_pool(name="psA", bufs=1, space="PSUM")

        # ---- phase A: projections, oc-pairs outer / d / l inner ----
        # oc: 0=k, 1..4=q heads 0..3, 5=v
        rope_dst = [krope] + qrope
        for ocp in range(3):
            oc_pair = (2 * ocp, 2 * ocp + 1)
            ps_tiles = {}
            for oi, oc in enumerate(oc_pair):
                for l in range(NLT):
                    slot = oi * NLT + l
                    ps_tiles[(oc, l)] = psA.tile(
                        [128, LT], f32, tag=f"pj{slot}", name=f"pj{slot}")
            for d in range(ND):
                for oc in oc_pair:
                    if oc == 0:
                        w_ap = wk_sb[:, d * 128:(d + 1) * 128]
                    elif oc < 5:
                        qh = oc - 1
                        w_ap = wq_sb[:, d * 512 + qh * 128:d * 512 + (qh + 1) * 128]
                    else:
                        w_ap = wv_sb[:, d * 128:(d + 1) * 128]
                    for l in range(NLT):
                        nc.tensor.matmul(
                            ps_tiles[(oc, l)][:], w_ap,
                            x_sb[:, d * L + l * LT: d * L + (l + 1) * LT],
                            start=(d == 0), stop=(d == ND - 1))
            # rope / copy-out for this pair
            for oi, oc in enumerate(oc_pair):
                for l in range(NLT):
                    lsl = slice(l * LT, (l + 1) * LT)
                    slot = oi * NLT + l
                    if oc == 5:
                        nc.scalar.copy(vrawT[:, lsl], ps_tiles[(oc, l)][:])
                        continue
                    qs = p_qs.tile([128, LT], bf16, tag="qs", name="qs")
                    nc.scalar.copy(qs[:], ps_tiles[(oc, l)][:])
                    # swap matmul reuses the same PSUM slot (after qs copy)
                    psw = psA.tile([128, LT], f32, tag=f"pj{slot}",
                                   name=f"psw{slot}")
                    nc.tensor.matmul(psw[:], pswap_sb[:], qs[:],
                                     start=True, stop=True)
                    t1 = p_tmp.tile([128, LT], f32, tag="t1", name="t1")
                    nc.vector.tensor_mul(t1[:], qs[:], cos_sb[:, lsl])
                    t2 = p_tmp.tile([128, LT], f32, tag="t2", name="t2")
                    nc.vector.tensor_mul(t2[:], psw[:], sin_sb[:, lsl])
                    nc.vector.tensor_add(rope_dst[oc][:, lsl], t1[:], t2[:])
        psA.release()

        # ---- v transpose: v_sb[lk, kdim] in 16 column chunks ----
        psVT = tc.alloc_tile_pool(name="psVT", bufs=2, space="PSUM")
        for c in range(L // 128):
            pvt = psVT.tile([128, 128], bf16, tag="vt", name="vt")
            nc.tensor.transpose(pvt[:], vrawT[:, c * 128:(c + 1) * 128], ident_sb[:])
            nc.scalar.copy(v_sb[:, c * 128:(c + 1) * 128], pvt[:])
        psVT.release()

        # ---- phases C+D fused per lq-tile ----
        psS = tc.alloc_tile_pool(name="psS", bufs=3, space="PSUM")
        psO = tc.alloc_tile_pool(name="psO", bufs=2, space="PSUM")
        psSUM = tc.alloc_tile_pool(name="psSUM", bufs=1, space="PSUM")
        psY = tc.alloc_tile_pool(name="psY", bufs=2, space="PSUM")
        for jq in range(NLT):
            qsl = slice(jq * LT, (jq + 1) * LT)
            onorm = []
            for h in range(NH):
                nch = 4 * (jq + 1)
                po = psO.tile([128, LT], f32, tag="po", name="po")
                acc = p_acc.tile([128, LT], bf16, tag="acc", name="acc")
                for c in range(nch):
                    ps = psS.tile([128, LT], f32, tag="ps", name="ps")
                    nc.tensor.matmul(ps[:], krope[:, c * 128:(c + 1) * 128],
                                     qrope[h][:, qsl], start=True, stop=True)
                    r = c - 4 * jq
                    if r >= 0:
                        w = 128 * (r + 1)
                        nc.vector.tensor_add(ps[:, 0:w], ps[:, 0:w],
                                             mask_sb[:, moff[r]:moff[r] + w])
                    pt = p_pt.tile([128, LT], bf16, tag="pt", name="pt")
                    nc.scalar.activation(pt[:], ps[:], EXP)
                    nc.tensor.matmul(po[:], v_sb[:, c * 128:(c + 1) * 128],
                                     pt[:], start=(c == 0),
                                     stop=(c == nch - 1), skip_group_check=True)
                    if c == 0:
                        nc.vector.tensor_copy(acc[:], pt[:])
                    else:
                        nc.vector.tensor_add(acc[:], acc[:], pt[:])
                psm = psSUM.tile([1, LT], f32, tag="pm", name="pm")
                nc.tensor.matmul(psm[:], ones_sb[:, 0:1], acc[:],
                                 start=True, stop=True)
                sm = p_rc.tile([1, LT], f32, tag="rc", name="rc")
                nc.vector.tensor_copy(sm[:], psm[:])
                bs = p_bc.tile([128, LT], f32, tag="bs", name="bs")
                nc.gpsimd.partition_broadcast(bs[:], sm[:])
                bc = p_bc.tile([128, LT], f32, tag="bc", name="bc")
                nc.vector.reciprocal_approx_fast(bc[:], bs[:])
                on = p_on.tile([128, LT], bf16, tag=f"on{h}", name=f"on{h}")
                nc.vector.tensor_mul(on[:], po[:], bc[:])
                onorm.append(on)
            # output projection for this lq-tile (resident wo)
            for j in range(NJ):
                py = psY.tile([128, LT], f32, tag="py", name="py")
                for h in range(NH):
                    nc.tensor.matmul(
                        py[:], wo_sb[:, (h * NJ + j) * 128:(h * NJ + j + 1) * 128],
                        onorm[h][:], start=(h == 0), stop=(h == NH - 1))
                yt = p_ysb.tile([128, LT], bf16, tag="yt", name="yt")
                if j % 2 == 0:
                    nc.vector.tensor_copy(yt[:], py[:])
                else:
                    nc.scalar.copy(yt[:], py[:])
                nc.sync.dma_start(out=yT.ap()[j, jq], in_=yt[:])
        psY.release()
        psSUM.release()
        psO.release()
        psS.release()
        for pool in (p_ysb, p_bc, p_rc, p_on, p_acc, p_pt, p_tmp, p_qs,
                     p_rope, p_vraw, p_x, p_wkv, p_const):
            pool.release()

    nc.compile()
    return nc


def _get_nc():
    if "nc" not in _NC_CACHE:
        import concourse.mybir as mybir  # noqa: F401
        _NC_CACHE["nc"] = _build_nc()
    return _NC_CACHE["nc"]


def _host_tables():
    import ml_dtypes
    bf = ml_dtypes.bfloat16
    iv = (1.0 / (ROPE_BASE ** (np.arange(0, K, 2, dtype=np.float32) / np.float32(K)))).astype(np.float32)
    t = np.arange(L, dtype=np.float32)
    freqs = np.outer(t, iv).astype(np.float32)          # [L, 64]
    cos = np.cos(freqs).astype(np.float32)
    sin = np.sin(freqs).astype(np.float32)
    cosT = np.empty((128, L), np.float32)
    sinT = np.empty((128, L), np.float32)
    cosT[0::2] = cos.T
    cosT[1::2] = cos.T
    sinT[0::2] = -sin.T
    sinT[1::2] = sin.T

    p = np.arange(128)[:, None]
    f = np.arange(LT)[None, :]
    masks = np.zeros((4, 128, LT), np.float32)
    for r in range(4):
        masks[r] = np.where(f < 128 * r + p, np.float32(MASK_VAL), np.float32(0.0))

    pswap = np.zeros((128, 128), np.float32)
    idx = np.arange(128)
    pswap[idx ^ 1, idx] = 1.0
    onesc = np.ones((128, 8), np.float32)
    ident = np.eye(128, dtype=np.float32)
    return (cosT, sinT, masks, pswap.astype(bf), onesc.astype(bf),
            ident.astype(bf))


def _tile_xT(xb, bf):
    # x[b] [L, D] -> xT tiles [ND, 128, L]: xT[d, l] = x[l, d]
    xT = xb.T.astype(bf)  # [D, L]
    return np.ascontiguousarray(xT.reshape(ND, 128, L))


def _prep_inputs(x, wq, wk, wv, wo):
    import ml_dtypes
    bf = ml_dtypes.bfloat16
    cosT, sinT, masks, pswap, onesc, ident = _host_tables()
    scale = np.float32(K) ** np.float32(-0.5)
    in_maps = []
    xts = [_tile_xT(np.ascontiguousarray(x[b]), bf) for b in range(B)]
    for b in range(B):
        for g in range(KV):
            wq_g = (wq[g * 512:(g + 1) * 512, :] * scale).astype(bf)
            wqT_t = np.ascontiguousarray(
                wq_g.T.reshape(ND, 128, 512))                      # [d, 128, 512]
            wk_g = wk[g * 128:(g + 1) * 128, :].astype(bf)
            wkT_t = np.ascontiguousarray(wk_g.T.reshape(ND, 128, 128))
            wv_g = wv[g * 128:(g + 1) * 128, :].astype(bf)
            wvT_t = np.ascontiguousarray(wv_g.T.reshape(ND, 128, 128))
            wo_g = wo[:, g * 512:(g + 1) * 512]                    # [D, 512]
            # woT flat [128, (h*NJ+j)*128 + c] = wo[128j+c, 512g+128h+p]
            woT_t = np.ascontiguousarray(
                wo_g.T.reshape(NH, 128, NJ, 128).transpose(1, 0, 2, 3)
                .reshape(128, NH * NJ * 128)).astype(bf)
            in_maps.append({
                "xT": xts[b], "wqT": wqT_t, "wkT": wkT_t, "wvT": wvT_t,
                "woT": woT_t, "cosT": cosT, "sinT": sinT, "masks": masks,
                "pswap": pswap, "onesc": onesc, "ident": ident,
            })
    return in_maps


def _gather(results):
    out = np.empty((B, L, D), np.float32)
    for b in range(B):
        acc = None
        for g in range(KV):
            yt = results[b * KV + g]["yT"].astype(np.float32)  # [NJ, NLT, 128, LT]
            full = yt.transpose(0, 2, 1, 3).reshape(D, L)      # [j, l]
            acc = full if acc is None else acc + full
        out[b] = acc.T
    return out


def run(inputs, trace=False, trace_kwargs=None):
    from concourse.bass_utils import run_bass_kernel_spmd
    nc = _get_nc()
    in_maps = _prep_inputs(**inputs)
    res = run_bass_kernel_spmd(nc, in_maps, list(range(8)), trace=trace,
                               **(trace_kwargs or {}))
    return _gather(res.results), res


def kernel(x, wq, wk, wv, wo):
    out, _ = run({"x": x, "wq": wq, "wk": wk, "wv": wv, "wo": wo})
    return out


# revision 24
# speedup vs baseline: 1.1561x; 1.0164x over previous
"""Causal GQA attention (B=2, L=2048, D=2048, H=16, KV=4, K=128) on 8 trn2 cores.

Sharding: core = b*4 + g  (b: batch 0..1, g: GQA group 0..3).
Each core computes, for its batch b and its 4 Q heads / 1 KV head:
    q/k/v projections -> rope -> causal attention -> partial out-projection
and writes yT_partial = (partial y).T to DRAM (bf16). Host sums the 4 group
partials per batch and transposes back.

v2 vs baseline:
 - all PE operands bf16 (halves DMA + SBUF traffic; LDW data)
 - x fully resident in SBUF; projections reordered (oc-pairs outer, d, l
   inner) so each weight-chunk stationary load serves 4 matmuls
 - wo resident (one DMA) instead of 256 per-jq wos DMAs
 - softmax row sums: DVE accumulates exp chunks, one ones-matmul per
   (head, lq-tile) instead of one per chunk (saves ~144 PE matmuls)
 - x DMA triggers on the (otherwise idle) gpsimd queue
"""

import sys

if "/opt/trn_rl_repo" not in sys.path:
    sys.path.insert(0, "/opt/trn_rl_repo")

import numpy as np

B, L, D, H, KV = 2, 2048, 2048, 16, 4
K = D // H          # 128 head dim
G = H // KV         # 4 q heads per kv head
NH = G              # q heads per core
LT = 512            # seq tile (moving operand width)
NLT = L // LT       # 4
ND = D // 128       # 16 contraction chunks
NJ = D // 128       # 16 output-column chunks
ROPE_BASE = 10000.0
MASK_VAL = -30000.0

_NC_CACHE = {}


def _build_nc():
    import concourse.bacc as bacc
    import concourse.mybir as mybir
    from concourse.tile import TileContext

    f32 = mybir.dt.float32
    f32r = mybir.dt.float32r
    bf16 = mybir.dt.bfloat16
    EXP = mybir.ActivationFunctionType.Exp
    nc = bacc.Bacc("TRN2", target_bir_lowering=False, debug=False, num_devices=8)

    # ---- DRAM parameters (host-pre-tiled layouts, bf16) ----
    xT = nc.dram_tensor("xT", [ND, 128, L], bf16, kind="ExternalInput")
    wqT = nc.dram_tensor("wqT", [ND, 128, 512], bf16, kind="ExternalInput")
    wkT = nc.dram_tensor("wkT", [ND, 128, 128], bf16, kind="ExternalInput")
    wvT = nc.dram_tensor("wvT", [ND, 128, 128], bf16, kind="ExternalInput")
    woT = nc.dram_tensor("woT", [128, NH * NJ * 128], bf16, kind="ExternalInput")
    cosT = nc.dram_tensor("cosT", [128, L], f32, kind="ExternalInput")
    sinT = nc.dram_tensor("sinT", [128, L], f32, kind="ExternalInput")
    masks = nc.dram_tensor("masks", [4, 128, LT], f32, kind="ExternalInput")
    pswap = nc.dram_tensor("pswap", [128, 128], bf16, kind="ExternalInput")
    onesc = nc.dram_tensor("onesc", [128, 8], bf16, kind="ExternalInput")
    ident = nc.dram_tensor("ident", [128, 128], bf16, kind="ExternalInput")
    yT = nc.dram_tensor("yT", [NJ, NLT, 128, LT], bf16, kind="ExternalOutput")

    with TileContext(nc) as tc:
        p_const = tc.alloc_tile_pool(name="const", bufs=1)
        p_wkv = tc.alloc_tile_pool(name="wkv", bufs=1)
        p_x = tc.alloc_tile_pool(name="xres", bufs=1)
        p_vraw = tc.alloc_tile_pool(name="vraw", bufs=1)
        p_rope = tc.alloc_tile_pool(name="ropeout", bufs=1)
        p_qs = tc.alloc_tile_pool(name="qs", bufs=4)
        p_tmp = tc.alloc_tile_pool(name="tmp", bufs=4)
        p_pt = tc.alloc_tile_pool(name="pt", bufs=3)
        p_acc = tc.alloc_tile_pool(name="acc", bufs=2)
        p_on = tc.alloc_tile_pool(name="on", bufs=2)
        p_rc = tc.alloc_tile_pool(name="rc", bufs=2)
        p_bc = tc.alloc_tile_pool(name="bc", bufs=2)
        p_ysb = tc.alloc_tile_pool(name="ysb", bufs=3)

        # ---- constants / weights (prefetch on sync queue) ----
        cos_sb = p_const.tile([128, L], f32, tag="cos", name="cos")
        nc.sync.dma_start(out=cos_sb[:], in_=cosT.ap())
        sin_sb = p_const.tile([128, L], f32, tag="sin", name="sin")
        nc.sync.dma_start(out=sin_sb[:], in_=sinT.ap())
        pswap_sb = p_const.tile([128, 128], bf16, tag="pswap", name="pswap")
        nc.sync.dma_start(out=pswap_sb[:], in_=pswap.ap())
        ones_sb = p_const.tile([128, 8], bf16, tag="ones", name="ones")
        nc.sync.dma_start(out=ones_sb[:], in_=onesc.ap())
        ident_sb = p_const.tile([128, 128], bf16, tag="ident", name="ident")
        nc.sync.dma_start(out=ident_sb[:], in_=ident.ap())
        mask_sb = p_const.tile([128, 1280], f32, tag="mask", name="mask")
        moff = [0, 128, 384, 768]
        for r in range(4):
            w = 128 * (r + 1)
            nc.sync.dma_start(out=mask_sb[:, moff[r]:moff[r] + w],
                              in_=masks.ap()[r][:, 0:w])

        wk_sb = p_wkv.tile([128, ND * 128], bf16, tag="wk", name="wk")
        wv_sb = p_wkv.tile([128, ND * 128], bf16, tag="wv", name="wv")
        wq_sb = p_wkv.tile([128, ND * 512], bf16, tag="wq", name="wq")
        wo_sb = p_wkv.tile([128, NH * NJ * 128], bf16, tag="wo", name="wo")
        for d in range(ND):
            nc.sync.dma_start(out=wv_sb[:, d * 128:(d + 1) * 128],
                              in_=wvT.ap()[d])
            nc.scalar.dma_start(out=wk_sb[:, d * 128:(d + 1) * 128],
                                in_=wkT.ap()[d])
        for d in range(ND):
            nc.scalar.dma_start(out=wq_sb[:, d * 512:(d + 1) * 512],
                                in_=wqT.ap()[d])
        nc.sync.dma_start(out=wo_sb[:], in_=woT.ap())

        # ---- resident x: [128, d*2048 + l*512 + i], DMA'd d-major on gpsimd ----
        x_sb = p_x.tile([128, ND * L], bf16, tag="x", name="x")
        for d in range(ND):
            nc.gpsimd.dma_start(
                out=x_sb[:, d * L: (d + 1) * L], in_=xT.ap()[d])

        # rope outputs + v
        vrawT = p_vraw.tile([128, L], bf16, tag="vrawT", name="vrawT")
        qrope = [p_rope.tile([128, L], bf16, tag=f"qrope{h}", name=f"qrope{h}")
                 for h in range(NH)]
        krope = p_rope.tile([128, L], bf16, tag="krope", name="krope")
        v_sb = p_rope.tile([128, L], bf16, tag="v", name="v")

        # PSUM: psA holds 8 banks (2 oc-groups of 4 l-tiles)
        psA = tc.alloc_tile_pool(name="psA", bufs=1, space="PSUM")

        # ---- phase A: projections, oc-pairs outer / d / l inner ----
        # oc: 0=k, 1..4=q heads 0..3, 5=v
        rope_dst = [krope] + qrope
        for ocp, oc_pair in enumerate([(0, 1), (2, 3), (4, 5)]):
            ps_tiles = {}
            for oi, oc in enumerate(oc_pair):
                for l in range(NLT):
                    slot = oi * NLT + l
                    ps_tiles[(oc, l)] = psA.tile(
                        [128, LT], f32, tag=f"pj{slot}", name=f"pj{slot}")
            for d in range(ND):
                for oc in oc_pair:
                    if oc == 0:
                        w_ap = wk_sb[:, d * 128:(d + 1) * 128]
                    elif oc < 5:
                        qh = oc - 1
                        w_ap = wq_sb[:, d * 512 + qh * 128:d * 512 + (qh + 1) * 128]
                    else:
                        w_ap = wv_sb[:, d * 128:(d + 1) * 128]
                    for l in range(NLT):
                        nc.tensor.matmul(
                            ps_tiles[(oc, l)][:], w_ap,
                            x_sb[:, d * L + l * LT: d * L + (l + 1) * LT],
                            start=(d == 0), stop=(d == ND - 1))
            # rope / copy-out for this pair
            for oi, oc in enumerate(oc_pair):
                for l in range(NLT):
                    lsl = slice(l * LT, (l + 1) * LT)
                    slot = oi * NLT + l
                    if oc == 5:
                        nc.scalar.copy(vrawT[:, lsl], ps_tiles[(oc, l)][:])
                        continue
                    qs = p_qs.tile([128, LT], bf16, tag="qs", name="qs")
                    nc.scalar.copy(qs[:], ps_tiles[(oc, l)][:])
                    # swap matmul reuses the same PSUM slot (after qs copy)
                    psw = psA.tile([128, LT], f32, tag=f"pj{slot}",
                                   name=f"psw{slot}")
                    nc.tensor.matmul(psw[:], pswap_sb[:], qs[:],
                                     start=True, stop=True)
                    t1 = p_tmp.tile([128, LT], f32, tag="t1", name="t1")
                    nc.vector.tensor_mul(t1[:], qs[:], cos_sb[:, lsl])
                    t2 = p_tmp.tile([128, LT], f32, tag="t2", name="t2")
                    nc.vector.tensor_mul(t2[:], psw[:], sin_sb[:, lsl])
                    nc.vector.tensor_add(rope_dst[oc][:, lsl], t1[:], t2[:])
        psA.release()

        # ---- v transpose: v_sb[lk, kdim] in 16 column chunks ----
        psVT = tc.alloc_tile_pool(name="psVT", bufs=4, space="PSUM")
        for c in range(L // 128):
            pvt = psVT.tile([128, 128], bf16, tag="vt", name="vt")
            nc.tensor.transpose(pvt[:], vrawT[:, c * 128:(c + 1) * 128], ident_sb[:])
            nc.scalar.copy(v_sb[:, c * 128:(c + 1) * 128], pvt[:])
        psVT.release()

        # ---- phases C+D fused per lq-tile ----
        psS = tc.alloc_tile_pool(name="psS", bufs=3, space="PSUM")
        psO = tc.alloc_tile_pool(name="psO", bufs=2, space="PSUM")
        psSUM = tc.alloc_tile_pool(name="psSUM", bufs=1, space="PSUM")
        psY = tc.alloc_tile_pool(name="psY", bufs=2, space="PSUM")
        for jq in range(NLT):
            qsl = slice(jq * LT, (jq + 1) * LT)
            onorm = []
            for h in range(NH):
                nch = 4 * (jq + 1)
                po = psO.tile([128, LT], f32, tag="po", name="po")
                acc = p_acc.tile([128, LT], bf16, tag="acc", name="acc")
                for c in range(nch):
                    ps = psS.tile([128, LT], f32, tag="ps", name="ps")
                    nc.tensor.matmul(ps[:], krope[:, c * 128:(c + 1) * 128],
                                     qrope[h][:, qsl], start=True, stop=True)
                    r = c - 4 * jq
                    if r >= 0:
                        w = 128 * (r + 1)
                        nc.vector.tensor_add(ps[:, 0:w], ps[:, 0:w],
                                             mask_sb[:, moff[r]:moff[r] + w])
                    pt = p_pt.tile([128, LT], bf16, tag="pt", name="pt")
                    nc.scalar.activation(pt[:], ps[:], EXP)
                    nc.tensor.matmul(po[:], v_sb[:, c * 128:(c + 1) * 128],
                                     pt[:], start=(c == 0),
                                     stop=(c == nch - 1), skip_group_check=True)
                    if c == 0:
                        nc.vector.tensor_copy(acc[:], pt[:])
                    else:
                        nc.vector.tensor_add(acc[:], acc[:], pt[:])
                psm = psSUM.tile([1, LT], f32, tag="pm", name="pm")
                nc.tensor.matmul(psm[:], ones_sb[:, 0:1], acc[:],
                                 start=True, stop=True)
                sm = p_rc.tile([1, LT], f32, tag="rc", name="rc")
                nc.vector.tensor_copy(sm[:], psm[:])
                bs = p_bc.tile([128, LT], f32, tag="bs", name="bs")
                nc.gpsimd.partition_broadcast(bs[:], sm[:])
                bc = p_bc.tile([128, LT], f32, tag="bc", name="bc")
                nc.vector.reciprocal_approx_fast(bc[:], bs[:])
                on = p_on.tile([128, LT], bf16, tag=f"on{h}", name=f"on{h}")
                nc.vector.tensor_mul(on[:], po[:], bc[:])
                onorm.append(on)
            # output projection for this lq-tile (resident wo)
            for j in range(NJ):
                py = psY.tile([128, LT], f32, tag="py", name="py")
                for h in range(NH):
                    nc.tensor.matmul(
                        py[:], wo_sb[:, (h * NJ + j) * 128:(h * NJ + j + 1) * 128],
                        onorm[h][:], start=(h == 0), stop=(h == NH - 1))
                yt = p_ysb.tile([128, LT], bf16, tag="yt", name="yt")
                if j % 2 == 0:
                    nc.vector.tensor_copy(yt[:], py[:])
                else:
                    nc.scalar.copy(yt[:], py[:])
                nc.sync.dma_start(out=yT.ap()[j, jq], in_=yt[:])
        psY.release()
        psSUM.release()
        psO.release()
        psS.release()
        for pool in (p_ysb, p_bc, p_rc, p_on, p_acc, p_pt, p_tmp, p_qs,
                     p_rope, p_vraw, p_x, p_wkv, p_const):
            pool.release()

    nc.compile()
    return nc


def _get_nc():
    if "nc" not in _NC_CACHE:
        import concourse.mybir as mybir  # noqa: F401
        _NC_CACHE["nc"] = _build_nc()
    return _NC_CACHE["nc"]


def _host_tables():
    import ml_dtypes
    bf = ml_dtypes.bfloat16
    iv = (1.0 / (ROPE_BASE ** (np.arange(0, K, 2, dtype=np.float32) / np.float32(K)))).astype(np.float32)
    t = np.arange(L, dtype=np.float32)
    freqs = np.outer(t, iv).astype(np.float32)          # [L, 64]
    cos = np.cos(freqs).astype(np.float32)
    sin = np.sin(freqs).astype(np.float32)
    cosT = np.empty((128, L), np.float32)
    sinT = np.empty((128, L), np.float32)
    cosT[0::2] = cos.T
    cosT[1::2] = cos.T
    sinT[0::2] = -sin.T
    sinT[1::2] = sin.T

    p = np.arange(128)[:, None]
    f = np.arange(LT)[None, :]
    masks = np.zeros((4, 128, LT), np.float32)
    for r in range(4):
        masks[r] = np.where(f < 128 * r + p, np.float32(MASK_VAL), np.float32(0.0))

    pswap = np.zeros((128, 128), np.float32)
    idx = np.arange(128)
    pswap[idx ^ 1, idx] = 1.0
    onesc = np.ones((128, 8), np.float32)
    ident = np.eye(128, dtype=np.float32)
    return (cosT, sinT, masks, pswap.astype(bf), onesc.astype(bf),
            ident.astype(bf))


def _tile_xT(xb, bf):
    # x[b] [L, D] -> xT tiles [ND, 128, L]: xT[d, l] = x[l, d]
    xT = xb.T.astype(bf)  # [D, L]
    return np.ascontiguousarray(xT.reshape(ND, 128, L))


def _prep_inputs(x, wq, wk, wv, wo):
    import ml_dtypes
    bf = ml_dtypes.bfloat16
    cosT, sinT, masks, pswap, onesc, ident = _host_tables()
    scale = np.float32(K) ** np.float32(-0.5)
    in_maps = []
    xts = [_tile_xT(np.ascontiguousarray(x[b]), bf) for b in range(B)]
    for b in range(B):
        for g in range(KV):
            wq_g = (wq[g * 512:(g + 1) * 512, :] * scale).astype(bf)
            wqT_t = np.ascontiguousarray(
                wq_g.T.reshape(ND, 128, 512))                      # [d, 128, 512]
            wk_g = wk[g * 128:(g + 1) * 128, :].astype(bf)
            wkT_t = np.ascontiguousarray(wk_g.T.reshape(ND, 128, 128))
            wv_g = wv[g * 128:(g + 1) * 128, :].astype(bf)
            wvT_t = np.ascontiguousarray(wv_g.T.reshape(ND, 128, 128))
            wo_g = wo[:, g * 512:(g + 1) * 512]                    # [D, 512]
            # woT flat [128, (h*NJ+j)*128 + c] = wo[128j+c, 512g+128h+p]
            woT_t = np.ascontiguousarray(
                wo_g.T.reshape(NH, 128, NJ, 128).transpose(1, 0, 2, 3)
                .reshape(128, NH * NJ * 128)).astype(bf)
            in_maps.append({
                "xT": xts[b], "wqT": wqT_t, "wkT": wkT_t, "wvT": wvT_t,
                "woT": woT_t, "cosT": cosT, "sinT": sinT, "masks": masks,
                "pswap": pswap, "onesc": onesc, "ident": ident,
            })
    return in_maps


def _gather(results):
    out = np.empty((B, L, D), np.float32)
    for b in range(B):
        acc = None
        for g in range(KV):
            yt = results[b * KV + g]["yT"].astype(np.float32)  # [NJ, NLT, 128, LT]
            full = yt.transpose(0, 2, 1, 3).reshape(D, L)      # [j, l]
            acc = full if acc is None else acc + full
        out[b] = acc.T
    return out


def run(inputs, trace=False, trace_kwargs=None):
    from concourse.bass_utils import run_bass_kernel_spmd
    nc = _get_nc()
    in_maps = _prep_inputs(**inputs)
    res = run_bass_kernel_spmd(nc, in_maps, list(range(8)), trace=trace,
                               **(trace_kwargs or {}))
    return _gather(res.results), res


def kernel(x, wq, wk, wv, wo):
    out, _ = run({"x": x, "wq": wq, "wk": wk, "wv": wv, "wo": wo})
    return out
